# revision 9
# baseline (speedup 1.0000x reference)
"""ALiBi causal attention block on 8 TRN2 NeuronCores.

Sharding: core c -> (batch b = c//2, head-group g = c%2) ; 6 heads per group.
Each core computes qkv for its head group, causal attention (the ALiBi bias
max(col-row,0)*slope is identically zero on the causal region, so it drops
out), and a partial c_proj against its head-group's w_proj columns.
Host unshard: out[b] = partial[2b] + partial[2b+1].

Math notes:
 - softmax without max-subtraction: logits are ~N(0, 0.23^2) on the causal
   region, exp() is safe in f32 and mathematically identical to the reference.
 - QK^T / qkv / c_proj matmuls run as float32r (full-rate fp32 path).
 - exp output and PV matmul run in bf16 (error ~4e-3 << 2e-2 gate); PV
   accumulates in f32 PSUM. A ones-column appended to V gives the softmax
   denominator from the same matmul.
"""

import numpy as np
import ml_dtypes

import concourse.bass as bass
import concourse.mybir as mybir
import concourse.tile as tile
from concourse import bacc
from concourse.bass_utils import run_bass_kernel_spmd

B, T, C = 4, 2048, 576
H = 12               # total heads
HG = 6               # heads per core (head-group)
D = 48               # head dim
CG = HG * D          # 288 channels per group
NT = T // 128        # 16 row tiles
NB = T // 512        # 4  i-blocks of 512
SCALE = 1.0 / float(np.sqrt(D))

F32 = mybir.dt.float32
F32R = mybir.dt.float32r
BF16 = mybir.dt.bfloat16

# contraction chunks over C=576: 4x128 + 64
C_CHUNKS = [(0, 128), (128, 128), (256, 128), (384, 128), (512, 64)]
# contraction chunks over CG=288 for c_proj: 3x96
G_CHUNKS = [(0, 96), (96, 96), (192, 96)]


def r(ap):
    return ap.bitcast(F32R)


def build_nc():
    nc = bacc.Bacc("TRN2", target_bir_lowering=False, debug=False)

    xT_d = nc.dram_tensor("xT", [C, T], F32R, kind="ExternalInput")        # x[b].T
    wqkvT_d = nc.dram_tensor("wqkvT", [C, 3 * CG], F32R, kind="ExternalInput")
    wpT_d = nc.dram_tensor("wpT", [CG, C], F32R, kind="ExternalInput")
    mask_d = nc.dram_tensor("mask", [128, 128], BF16, kind="ExternalInput")
    ident_d = nc.dram_tensor("ident", [128, 128], F32, kind="ExternalInput")
    out_d = nc.dram_tensor("out", [T, C], F32, kind="ExternalOutput")

    with tile.TileContext(nc) as tc:
        with (
            tc.tile_pool(name="wp", bufs=3) as p_wp,
            tc.tile_pool(name="qk", bufs=12) as p_qk,
            tc.tile_pool(name="vb", bufs=16) as p_vb,
            tc.tile_pool(name="y", bufs=16) as p_y,
            tc.tile_pool(name="misc", bufs=1) as p_misc,
            tc.tile_pool(name="rs", bufs=4) as p_rs,
            tc.tile_pool(name="mm", bufs=4, space="PSUM") as p_mm,
            tc.tile_pool(name="sm", bufs=4, space="PSUM") as p_sm,
        ):
            # ---- load constants / inputs into SBUF ----
            mask_t = p_misc.tile([128, 128], BF16, tag="mask")
            nc.sync.dma_start(mask_t[:], mask_d[:, :])
            ident_t = p_misc.tile([128, 128], F32, tag="ident")
            nc.sync.dma_start(ident_t[:], ident_d[:, :])

            stk = __import__("contextlib").ExitStack()
            p_xt = stk.enter_context(tc.tile_pool(name="xt", bufs=5))
            p_wq = stk.enter_context(tc.tile_pool(name="wq", bufs=5))
            xt = []
            for i, (c0, cn) in enumerate(C_CHUNKS):
                t_ = p_xt.tile([128, T], F32R, tag="xt", name="xt")
                nc.sync.dma_start(t_[:cn, :], xT_d[c0:c0 + cn, :])
                xt.append(t_)
            wq = []
            for i, (c0, cn) in enumerate(C_CHUNKS):
                t_ = p_wq.tile([128, 3 * CG], F32R, tag="wq", name="wq")
                nc.sync.dma_start(t_[:cn, :], wqkvT_d[c0:c0 + cn, :])
                wq.append(t_)
            wp = []
            for i, (g0, gn) in enumerate(G_CHUNKS):
                t_ = p_wp.tile([96, C], F32R, tag="wp", name="wp")
                nc.sync.dma_start(t_[:], wpT_d[g0:g0 + gn, :])
                wp.append(t_)

            # ---- qkv: q,k into [128, T] tiles, head pair at partition 0 / 64
            # (matmul operand base partition must be 0, 32 or 64) ----
            # qkvT row space: q rows 0..287, k rows 288..575, v rows 576..863
            qk = []  # 12 tiles [64, T]: q0..q5, k0..k5 (one head each)
            for m in range(12):
                qk.append(p_qk.tile([64, T], F32R, tag="qk", name="qk"))
            for m in range(12):
                r0 = m * D
                for ib in range(NB):
                    ps = p_mm.tile([128, 512], F32, tag="mm", name="mm")
                    for ck, (c0, cn) in enumerate(C_CHUNKS):
                        nc.tensor.matmul(
                            ps[0:D, :],
                            (wq[ck][:cn, r0:r0 + D]),
                            (xt[ck][:cn, ib * 512:(ib + 1) * 512]),
                            start=(ck == 0), stop=(ck == len(C_CHUNKS) - 1),
                        )
                    sl = slice(ib * 512, (ib + 1) * 512)
                    nc.scalar.copy(qk[m][0:D, sl], ps[0:D, :])

            # ---- v: route B -> [128 t, 288] per t-tile, cast to bf16 with
            #      a ones column per head: vb tile [128, 6*49] ----
            vb = []
            for it in range(NT):
                vt = p_vb.tile([128, HG * (D + 1)], BF16, tag="vb", name="vb")
                ps = p_mm.tile([128, 512], F32, tag="mm", name="mm")
                for ck, (c0, cn) in enumerate(C_CHUNKS):
                    nc.tensor.matmul(
                        ps[:, :CG],
                        (xt[ck][:cn, it * 128:(it + 1) * 128]),
                        (wq[ck][:cn, 2 * CG:3 * CG]),
                        start=(ck == 0), stop=(ck == len(C_CHUNKS) - 1),
                    )
                dst = vt[:, :].rearrange("p (h x) -> p h x", x=D + 1)
                nc.vector.tensor_copy(
                    dst[:, :, 0:D],
                    ps[:, :CG].rearrange("p (h d) -> p h d", d=D),
                )
                nc.vector.memset(dst[:, :, D:D + 1], 1.0)
                vb.append(vt)

            stk.close()  # free xt/wq SBUF for phase B pools
            stk2 = __import__("contextlib").ExitStack()
            p_exp = stk2.enter_context(tc.tile_pool(name="expt", bufs=17))
            p_yt = stk2.enter_context(tc.tile_pool(name="yt", bufs=3))
            p_osb = stk2.enter_context(tc.tile_pool(name="osb", bufs=2))
            # ---- attention per head ----
            y = []
            for it in range(NT):
                y.append(p_y.tile([128, CG], F32, tag="y", name="y"))

            for h in range(HG):
                qt = qk[h]
                kt = qk[6 + h]
                off = 0
                for ib in range(NB):
                    njt = 4 * ib + 4
                    etiles = []
                    for jt in range(njt):
                        diag_o = jt - 4 * ib          # >=0: j-tile inside i-block
                        lo = max(diag_o, 0) * 128     # local col start
                        ps = p_mm.tile([128, 512], F32, tag="mm", name="mm")
                        et = p_exp.tile([128, 512], BF16, tag="expt", name="expt")
                        nc.tensor.matmul(
                            ps[:, lo:512],
                            (kt[off:off + D, jt * 128:(jt + 1) * 128]),
                            (qt[off:off + D, ib * 512 + lo:(ib + 1) * 512]),
                            start=True, stop=True,
                        )
                        nc.scalar.activation(
                            et[:, lo:512], ps[:, lo:512],
                            mybir.ActivationFunctionType.Exp, scale=SCALE,
                        )
                        if diag_o >= 0:
                            nc.vector.tensor_mul(
                                et[:, lo:lo + 128], et[:, lo:lo + 128], mask_t[:]
                            )
                        etiles.append(et)
                    for o in range(4):
                        itg = 4 * ib + o
                        yp = p_sm.tile([128, D + 1], F32, tag="sm", name="sm")
                        for jt in range(itg + 1):
                            nc.tensor.matmul(
                                yp[:, :],
                                etiles[jt][:, o * 128:(o + 1) * 128],
                                vb[jt][:, h * (D + 1):(h + 1) * (D + 1)],
                                start=(jt == 0), stop=(jt == itg),
                            )
                        rs = p_rs.tile([128, 1], F32, tag="rs", name="rs")
                        nc.vector.reciprocal(rs[:], yp[:, D:D + 1])
                        nc.vector.tensor_scalar_mul(
                            y[itg][:, h * D:(h + 1) * D], yp[:, :D], rs[:]
                        )

            # ---- transpose y -> yT [96, T] x3 ----
            yt = []
            for m in range(3):
                yt.append(p_yt.tile([96, T], F32R, tag="yt", name="yt"))
            for it in range(NT):
                for m, (g0, gn) in enumerate(G_CHUNKS):
                    ps = p_sm.tile([128, 128], F32, tag="sm", name="sm")
                    nc.tensor.transpose(
                        ps[:96, :], y[it][:, g0:g0 + gn], ident_t[:]
                    )
                    nc.scalar.copy(yt[m][:, it * 128:(it + 1) * 128], ps[:96, :])

            # ---- c_proj partial: out[i, :] = sum_g yT[g, i]^T wpT[g, :] ----
            for it in range(NT):
                ob = p_osb.tile([128, C], F32, tag="osb", name="osb")
                for nb in range(2):
                    ps = p_sm.tile([128, CG], F32, tag="sm", name="sm")
                    for m in range(3):
                        nc.tensor.matmul(
                            ps[:, :],
                            (yt[m][:, it * 128:(it + 1) * 128]),
                            (wp[m][:, nb * CG:(nb + 1) * CG]),
                            start=(m == 0), stop=(m == 2),
                        )
                    nc.scalar.copy(ob[:, nb * CG:(nb + 1) * CG], ps[:, :])
                nc.sync.dma_start(out_d[it * 128:(it + 1) * 128, :], ob[:, :])

            stk2.close()

    nc.compile()
    return nc


def make_in_maps(x, w_qkv, w_proj):
    mask = np.triu(np.ones((128, 128), np.float32)).astype(ml_dtypes.bfloat16)
    ident = np.eye(128, dtype=np.float32)
    in_maps = []
    for c in range(8):
        b, g = c // 2, c % 2
        xT = np.ascontiguousarray(x[b].T).astype(np.float32)
        w = np.concatenate(
            [w_qkv[s * C + g * CG:s * C + (g + 1) * CG] for s in range(3)], 0
        )  # [864, 576]
        wqkvT = np.ascontiguousarray(w.T).astype(np.float32)
        wpT = np.ascontiguousarray(w_proj[:, g * CG:(g + 1) * CG].T).astype(
            np.float32
        )
        in_maps.append(
            {"xT": xT, "wqkvT": wqkvT, "wpT": wpT, "mask": mask, "ident": ident}
        )
    return in_maps


_NC_CACHE = {}


def _run(x, w_qkv, w_proj, trace=False):
    if "nc" not in _NC_CACHE:
        _NC_CACHE["nc"] = build_nc()
    nc = _NC_CACHE["nc"]
    in_maps = make_in_maps(x, w_qkv, w_proj)
    res = run_bass_kernel_spmd(nc, in_maps, core_ids=list(range(8)), trace=trace)
    outs = [res.results[c]["out"] for c in range(8)]
    full = np.stack([outs[2 * b] + outs[2 * b + 1] for b in range(B)], 0)
    return full.astype(np.float32), res


def kernel(x, w_qkv, w_proj):
    x = np.asarray(x, np.float32)
    w_qkv = np.asarray(w_qkv, np.float32)
    w_proj = np.asarray(w_proj, np.float32)
    out, _ = _run(x, w_qkv, w_proj, trace=False)
    return out


# revision 13
# speedup vs baseline: 1.1605x; 1.1605x over previous
"""ALiBi causal attention block on 8 TRN2 NeuronCores.

Sharding: core c -> (batch b = c//2, head-group g = c%2) ; 6 heads per group.
Each core computes qkv for its head group, causal attention (the ALiBi bias
max(col-row,0)*slope is identically zero on the causal region, so it drops
out), and a partial c_proj against its head-group's w_proj columns.
Host unshard: out[b] = partial[2b] + partial[2b+1].

Math notes:
 - softmax without max-subtraction: logits are ~N(0, 0.23^2) on the causal
   region, exp() is safe in f32 and mathematically identical to the reference.
 - QK^T / qkv / c_proj matmuls run as float32r (full-rate fp32 path).
 - exp output and PV matmul run in bf16 (error ~4e-3 << 2e-2 gate); PV
   accumulates in f32 PSUM. A ones-column appended to V gives the softmax
   denominator from the same matmul.
"""

import numpy as np
import ml_dtypes

import concourse.bass as bass
import concourse.mybir as mybir
import concourse.tile as tile
from concourse import bacc
from concourse.bass_utils import run_bass_kernel_spmd

B, T, C = 4, 2048, 576
H = 12               # total heads
HG = 6               # heads per core (head-group)
D = 48               # head dim
CG = HG * D          # 288 channels per group
NT = T // 128        # 16 row tiles
NB = T // 512        # 4  i-blocks of 512
SCALE = 1.0 / float(np.sqrt(D))

F32 = mybir.dt.float32
F32R = mybir.dt.float32r
BF16 = mybir.dt.bfloat16

# contraction chunks over C=576: 4x128 + 64
C_CHUNKS = [(0, 128), (128, 128), (256, 128), (384, 128), (512, 64)]
# contraction chunks over CG=288 for c_proj: 3x96
G_CHUNKS = [(0, 96), (96, 96), (192, 96)]


def r(ap):
    return ap.bitcast(F32R)


def build_nc():
    nc = bacc.Bacc("TRN2", target_bir_lowering=False, debug=False)

    xT_d = nc.dram_tensor("xT", [C, T], F32R, kind="ExternalInput")        # x[b].T
    wqkvT_d = nc.dram_tensor("wqkvT", [C, 3 * CG], F32R, kind="ExternalInput")
    wpT_d = nc.dram_tensor("wpT", [CG, C], F32R, kind="ExternalInput")
    mask_d = nc.dram_tensor("mask", [128, 128], BF16, kind="ExternalInput")
    ident_d = nc.dram_tensor("ident", [128, 128], F32, kind="ExternalInput")
    out_d = nc.dram_tensor("out", [T, C], F32, kind="ExternalOutput")

    with tile.TileContext(nc) as tc:
        with (
            tc.tile_pool(name="wp", bufs=3) as p_wp,
            tc.tile_pool(name="qk", bufs=12) as p_qk,
            tc.tile_pool(name="vb", bufs=16) as p_vb,
            tc.tile_pool(name="y", bufs=16) as p_y,
            tc.tile_pool(name="misc", bufs=1) as p_misc,
            tc.tile_pool(name="rs", bufs=4) as p_rs,
            tc.tile_pool(name="expt", bufs=17) as p_exp,
            tc.tile_pool(name="mm", bufs=5, space="PSUM") as p_mm,
            tc.tile_pool(name="sm", bufs=3, space="PSUM") as p_sm,
        ):
            # ---- load constants / inputs into SBUF ----
            mask_t = p_misc.tile([128, 128], BF16, tag="mask")
            nc.sync.dma_start(mask_t[:], mask_d[:, :])
            ident_t = p_misc.tile([128, 128], F32, tag="ident")
            nc.sync.dma_start(ident_t[:], ident_d[:, :])

            stk = __import__("contextlib").ExitStack()
            p_xt = stk.enter_context(tc.tile_pool(name="xt", bufs=5))
            p_wq = stk.enter_context(tc.tile_pool(name="wq", bufs=5))
            xt = []
            for i, (c0, cn) in enumerate(C_CHUNKS):
                t_ = p_xt.tile([128, T], F32R, tag="xt", name="xt")
                nc.sync.dma_start(t_[:cn, :], xT_d[c0:c0 + cn, :])
                xt.append(t_)
            wq = []
            for i, (c0, cn) in enumerate(C_CHUNKS):
                t_ = p_wq.tile([128, 3 * CG], F32R, tag="wq", name="wq")
                nc.sync.dma_start(t_[:cn, :], wqkvT_d[c0:c0 + cn, :])
                wq.append(t_)
            wp = []
            for i, (g0, gn) in enumerate(G_CHUNKS):
                t_ = p_wp.tile([96, C], F32R, tag="wp", name="wp")
                nc.sync.dma_start(t_[:], wpT_d[g0:g0 + gn, :])
                wp.append(t_)

            # ---- qkv: q,k into [128, T] tiles, head pair at partition 0 / 64
            # (matmul operand base partition must be 0, 32 or 64) ----
            # qkvT row space: q rows 0..287, k rows 288..575, v rows 576..863
            vb = []
            for it in range(NT):
                vt = p_vb.tile([128, HG * (D + 1)], BF16, tag="vb", name="vb")
                ps = p_mm.tile([128, 512], F32, tag="mm", name="mm")
                for ck, (c0, cn) in enumerate(C_CHUNKS):
                    nc.tensor.matmul(
                        ps[:, :CG],
                        (xt[ck][:cn, it * 128:(it + 1) * 128]),
                        (wq[ck][:cn, 2 * CG:3 * CG]),
                        start=(ck == 0), stop=(ck == len(C_CHUNKS) - 1),
                    )
                dst = vt[:, :].rearrange("p (h x) -> p h x", x=D + 1)
                nc.vector.tensor_copy(
                    dst[:, :, 0:D],
                    ps[:, :CG].rearrange("p (h d) -> p h d", d=D),
                )
                nc.vector.memset(dst[:, :, D:D + 1], 1.0)
                vb.append(vt)

            qk = []  # 12 tiles [64, T] bf16: q0..q5, k0..k5
            for m in range(12):
                qk.append(p_qk.tile([64, T], BF16, tag="qk", name="qk"))
            for h in range(HG):
                for m in (h, 6 + h):      # q then k of head h
                    r0 = m * D
                    for ib in range(NB):
                        ps = p_mm.tile([128, 512], F32, tag="mm", name="mm")
                        for ck, (c0, cn) in enumerate(C_CHUNKS):
                            nc.tensor.matmul(
                                ps[0:D, :],
                                (wq[ck][:cn, r0:r0 + D]),
                                (xt[ck][:cn, ib * 512:(ib + 1) * 512]),
                                start=(ck == 0), stop=(ck == len(C_CHUNKS) - 1),
                            )
                        sl = slice(ib * 512, (ib + 1) * 512)
                        nc.scalar.copy(qk[m][0:D, sl], ps[0:D, :])

            # ---- v: route B -> [128 t, 288] per t-tile, cast to bf16 with
            #      a ones column per head: vb tile [128, 6*49] ----

            stk.close()  # free xt/wq SBUF for phase B pools
            stk2 = __import__("contextlib").ExitStack()
            p_yt = stk2.enter_context(tc.tile_pool(name="yt", bufs=6))
            p_osb = stk2.enter_context(tc.tile_pool(name="osb", bufs=2))
            # ---- attention per head ----
            y = []
            for it in range(NT):
                y.append(p_y.tile([128, CG], F32, tag="y", name="y"))

            for ib in range(NB):
                for h in range(HG):
                    qt = qk[h]
                    kt = qk[6 + h]
                    off = 0
                    njt = 4 * ib + 4
                    etiles = []
                    for jt in range(njt):
                        diag_o = jt - 4 * ib          # >=0: j-tile inside i-block
                        lo = max(diag_o, 0) * 128     # local col start
                        ps = p_mm.tile([128, 512], F32, tag="mm", name="mm")
                        et = p_exp.tile([128, 512], BF16, tag="expt", name="expt")
                        nc.tensor.matmul(
                            ps[:, lo:512],
                            (kt[off:off + D, jt * 128:(jt + 1) * 128]),
                            (qt[off:off + D, ib * 512 + lo:(ib + 1) * 512]),
                            start=True, stop=True,
                        )
                        nc.scalar.activation(
                            et[:, lo:512], ps[:, lo:512],
                            mybir.ActivationFunctionType.Exp, scale=SCALE,
                        )
                        if diag_o >= 0:
                            nc.vector.tensor_mul(
                                et[:, lo:lo + 128], et[:, lo:lo + 128], mask_t[:]
                            )
                        etiles.append(et)
                    for o in range(4):
                        itg = 4 * ib + o
                        yp = p_sm.tile([128, D + 1], F32, tag="sm", name="sm")
                        for jt in range(itg + 1):
                            nc.tensor.matmul(
                                yp[:, :],
                                etiles[jt][:, o * 128:(o + 1) * 128],
                                vb[jt][:, h * (D + 1):(h + 1) * (D + 1)],
                                start=(jt == 0), stop=(jt == itg),
                            )
                        rs = p_rs.tile([128, 1], F32, tag="rs", name="rs")
                        nc.vector.reciprocal(rs[:], yp[:, D:D + 1])
                        nc.vector.tensor_scalar_mul(
                            y[itg][:, h * D:(h + 1) * D], yp[:, :D], rs[:]
                        )

                # fused tail for this i-block: transpose y -> yT, c_proj, DMA out
                for o in range(4):
                    it = 4 * ib + o
                    ytl = []
                    for m, (g0, gn) in enumerate(G_CHUNKS):
                        tp = p_sm.tile([128, 128], F32, tag="sm", name="tp")
                        nc.tensor.transpose(
                            tp[:96, :], y[it][:, g0:g0 + gn], ident_t[:]
                        )
                        ytt = p_yt.tile([96, 128], F32R, tag="yt", name="ytt")
                        nc.scalar.copy(ytt[:, :], tp[:96, :])
                        ytl.append(ytt)
                    ob = p_osb.tile([128, C], F32, tag="osb", name="osb")
                    for nb in range(2):
                        ps = p_sm.tile([128, CG], F32, tag="sm", name="sm")
                        for m in range(3):
                            nc.tensor.matmul(
                                ps[:, :],
                                (ytl[m][:, :]),
                                (wp[m][:, nb * CG:(nb + 1) * CG]),
                                start=(m == 0), stop=(m == 2),
                            )
                        nc.scalar.copy(ob[:, nb * CG:(nb + 1) * CG], ps[:, :])
                    nc.sync.dma_start(out_d[it * 128:(it + 1) * 128, :], ob[:, :])

            stk2.close()

    nc.compile()
    return nc


def make_in_maps(x, w_qkv, w_proj):
    mask = np.triu(np.ones((128, 128), np.float32)).astype(ml_dtypes.bfloat16)
    ident = np.eye(128, dtype=np.float32)
    in_maps = []
    for c in range(8):
        b, g = c // 2, c % 2
        xT = np.ascontiguousarray(x[b].T).astype(np.float32)
        w = np.concatenate(
            [w_qkv[s * C + g * CG:s * C + (g + 1) * CG] for s in range(3)], 0
        )  # [864, 576]
        wqkvT = np.ascontiguousarray(w.T).astype(np.float32)
        wpT = np.ascontiguousarray(w_proj[:, g * CG:(g + 1) * CG].T).astype(
            np.float32
        )
        in_maps.append(
            {"xT": xT, "wqkvT": wqkvT, "wpT": wpT, "mask": mask, "ident": ident}
        )
    return in_maps


_NC_CACHE = {}


def _run(x, w_qkv, w_proj, trace=False):
    if "nc" not in _NC_CACHE:
        _NC_CACHE["nc"] = build_nc()
    nc = _NC_CACHE["nc"]
    in_maps = make_in_maps(x, w_qkv, w_proj)
    res = run_bass_kernel_spmd(nc, in_maps, core_ids=list(range(8)), trace=trace)
    outs = [res.results[c]["out"] for c in range(8)]
    full = np.stack([outs[2 * b] + outs[2 * b + 1] for b in range(B)], 0)
    return full.astype(np.float32), res


def kernel(x, w_qkv, w_proj):
    x = np.asarray(x, np.float32)
    w_qkv = np.asarray(w_qkv, np.float32)
    w_proj = np.asarray(w_proj, np.float32)
    out, _ = _run(x, w_qkv, w_proj, trace=False)
    return out
